# revision 6
# baseline (speedup 1.0000x reference)
"""Trainium2 Bass kernel for nn_DeepCrossNetworkModel_Controller_hard.

Model: per-field embedding gather -> BatchNorm1d(F) (eval) -> controller
linear + softmax over fields -> top-k mask (renormalized) -> CrossNetwork(6)
+ MLP(2496->1024->512, BN+ReLU) -> concat -> linear -> sigmoid.

Strategy (data-parallel over 8 NeuronCores, 2048 rows each):
 - BN *scale* folded into the embedding table on host (bf16, rows padded to
   128 so dma_gather(transpose=True) lands embeddings feature-major).  The
   BN *offset* (beta - mean*scale, constant per field) is carried exactly
   through rank-39 correction matmuls (T0/Tu/c_ctrl), which lets the bulk
   MLP0/U matmuls run in fp8 DoubleRow (2 contraction planes per pass)
   without precision loss: fp8 only sees the small varying part.
 - All 156 gathers (39 fields x 4 blocks, 512 idxs each - the HW cap) are
   issued upfront; they serialize on the GPSIMD engine (~1.3us each), so
   the gather stream paces the pipeline and everything else hides under it.
 - Controller computed as z^T = Wc^T @ flat (20 N=512 matmuls/block,
   accumulated in PSUM as gathers land) + PE transpose to batch-major.
 - top-k of softmax + renormalize == softmax restricted to top-k logits.
 - CrossNetwork collapses algebraically: on device only U = x0 @ wu
   (7 cols) + a scalar recursion on [128,4] tiles.
"""

import sys

if "/opt/trn_rl_repo" not in sys.path:
    sys.path.insert(0, "/opt/trn_rl_repo")

import ml_dtypes
import numpy as np

import concourse.bass as bass
import concourse.bacc as bacc
import concourse.mybir as mybir
import concourse.tile as tile
from concourse.bass_utils import run_bass_kernel_spmd
from concourse.masks import make_identity

# Problem constants (hardcoded per spec).
B, F, E, L = 16384, 39, 64, 6
VOCAB = 10000
D = F * E  # 2496
H0, H1 = 1024, 512
EPS = 1e-5
NCORES = 8
BPC = B // NCORES      # 2048 rows per core
BLK = 512              # batch block
NBLK = BPC // BLK      # 4
NCHUNK = BLK // 128    # 4 chunks of 128 rows per block
KT = 20                # feature k-tiles of 128 (D padded 2496 -> 2560)
GD = KT // 2           # 10 fp8 DoubleRow double-tiles (256 contraction)
M0 = H0 // 128         # 8
M1 = H1 // 128         # 4
KT1 = H0 // 128        # 8
NQ = 4                 # SWDGE queues
SCALE_X = 256.0        # fp8 activation scale (baked into expand matrix S)
SCALE_W = 64.0         # fp8 weight scale
DESCALE = 1.0 / (SCALE_X * SCALE_W)   # 2^-14
# idx widths (wrapped-by-16 free dims): 39 single-field gathers of 512
# (transpose-mode dma_gather hangs on HW above 512 idxs per call)
IDXW_F = 512 // 16       # 32
IDXW_TOT = 39 * IDXW_F   # 1248: 20 even-field (direct) + 19 odd (scratch)

dt = mybir.dt
AF = mybir.ActivationFunctionType
OP = mybir.AluOpType
bf16 = ml_dtypes.bfloat16
f8 = ml_dtypes.float8_e4m3

_CACHE = {}


def _build(k, v_consts, c0, queue_map=None):
    """Build the per-core SPMD bass module.

    queue_map: optional {(blk, g, j): queue_num} overriding the default
    round-robin assignment, used to realign SWDGE queues with the
    scheduler's mod-8 DMASW semaphore rotation (see _build_aligned).
    """
    queue_map = queue_map or {}
    gather_insts = {}
    nc = bacc.Bacc("TRN2", target_bir_lowering=False, debug=False,
                   num_devices=NCORES, num_swdge_queues=NQ)
    nc._gather_insts = gather_insts

    idxs_d = nc.declare_dram_parameter("idxs", [NBLK, 128, IDXW_TOT], dt.int16, isOutput=False)
    tab_d = nc.declare_dram_parameter("tab", [F * VOCAB, 128], dt.bfloat16, isOutput=False)
    wc_d = nc.declare_dram_parameter("wc", [128, KT * F], dt.bfloat16, isOutput=False)
    cb_d = nc.declare_dram_parameter("cb", [F, 1], dt.float32, isOutput=False)
    w0_d = nc.declare_dram_parameter("w0", [128, GD * M0 * 256], dt.float8e4, isOutput=False)
    t0_d = nc.declare_dram_parameter("t0", [F, H0], dt.bfloat16, isOutput=False)
    w1_d = nc.declare_dram_parameter("w1", [128, KT1 * M1 * 128], dt.bfloat16, isOutput=False)
    wu_d = nc.declare_dram_parameter("wu", [128, GD * 32], dt.float8e4, isOutput=False)
    tu_d = nc.declare_dram_parameter("tu", [F, 7], dt.bfloat16, isOutput=False)
    s_d = nc.declare_dram_parameter("s", [F, KT * 128], dt.bfloat16, isOutput=False)
    lw2_d = nc.declare_dram_parameter("lw2", [128, M1], dt.bfloat16, isOutput=False)
    b0_d = nc.declare_dram_parameter("b0", [128, M0], dt.float32, isOutput=False)
    b1_d = nc.declare_dram_parameter("b1", [128, M1], dt.float32, isOutput=False)
    out_d = nc.declare_dram_parameter("out", [BPC], dt.float32, isOutput=True)

    rounds = -(-k // 8)  # ceil(k/8) match_replace rounds

    with tile.TileContext(nc) as tc:
        with (
            tc.tile_pool(name="const", bufs=1) as cpool,
            tc.tile_pool(name="flat", bufs=1) as flatp,
            tc.tile_pool(name="x0", bufs=2) as x0p,
            tc.tile_pool(name="big", bufs=2) as bigp,
            tc.tile_pool(name="msk", bufs=2) as mskp,
            tc.tile_pool(name="zt", bufs=2) as ztp,
            tc.tile_pool(name="gat", bufs=12) as gatp,
            tc.tile_pool(name="scr", bufs=8) as scr,
            # PSUM slots are bank-granular (8 banks x 2KB/partition):
            # psb x3 (ex/up/hp) + pzt x1 + pza x1 + pmt x1 + pss x2
            tc.tile_pool(name="psb", bufs=3, space="PSUM") as psb,
            tc.tile_pool(name="pzt", bufs=1, space="PSUM") as pzt,
            tc.tile_pool(name="pza", bufs=1, space="PSUM") as pza,
            tc.tile_pool(name="pmt", bufs=1, space="PSUM") as pmt,
            tc.tile_pool(name="pss", bufs=2, space="PSUM") as pss,
        ):
            # ---- identities + idx DMAs first (cheap, unblock gathers) ----
            idf = cpool.tile([128, 128], dt.float32)
            make_identity(nc, idf[:])
            idb = cpool.tile([128, 128], dt.bfloat16)
            make_identity(nc, idb[:])

            idx_sb = []
            for b in range(NBLK):
                t = cpool.tile([128, IDXW_TOT], dt.int16, name=f"idx{b}")
                nc.sync.dma_start(t[:], idxs_d[b, :, :])
                idx_sb.append(t)

            # ---- weights on the scalar HWDGE queue (small consts first) ----
            wc_sb = cpool.tile([128, KT * F], dt.bfloat16)
            nc.scalar.dma_start(wc_sb[:], wc_d[:])
            cb_sb = cpool.tile([F, 1], dt.float32)
            nc.scalar.dma_start(cb_sb[:], cb_d[:])
            s_sb = cpool.tile([F, KT * 128], dt.bfloat16)
            nc.scalar.dma_start(s_sb[:], s_d[:])
            wu_sb = cpool.tile([128, GD * 32], dt.float8e4)
            nc.scalar.dma_start(wu_sb[:], wu_d[:])
            tu_sb = cpool.tile([F, 7], dt.bfloat16)
            nc.scalar.dma_start(tu_sb[:], tu_d[:])
            t0_sb = cpool.tile([F, H0], dt.bfloat16)
            nc.scalar.dma_start(t0_sb[:], t0_d[:])
            lw2_sb = cpool.tile([128, M1], dt.bfloat16)
            nc.scalar.dma_start(lw2_sb[:], lw2_d[:])
            b0_sb = cpool.tile([128, M0], dt.float32)
            nc.scalar.dma_start(b0_sb[:], b0_d[:])
            b1_sb = cpool.tile([128, M1], dt.float32)
            nc.scalar.dma_start(b1_sb[:], b1_d[:])
            w0_sb = cpool.tile([128, GD * M0 * 256], dt.float8e4)
            for q in range(4):
                qs = GD * M0 * 256 // 4
                nc.scalar.dma_start(w0_sb[:, q * qs : (q + 1) * qs],
                                    w0_d[:, q * qs : (q + 1) * qs])
            w1_sb = cpool.tile([128, KT1 * M1 * 128], dt.bfloat16)
            nc.scalar.dma_start(w1_sb[:], w1_d[:])

            # ---- persistent activations ----
            flat_fm = [flatp.tile([128, KT * BLK], dt.bfloat16, tag=f"f{i}",
                                  name=f"flat{i}")
                       for i in range(NBLK)]
            p_sb = cpool.tile([128, BPC // 128], dt.float32)

            nreg = nc.gpsimd.to_reg(512)
            qcount = [0]  # global SWDGE queue round-robin

            gts = {}
            masks = {}
            mt_fms = {}
            zts = {}
            x0s = {}
            h0s = {}
            h1s = {}
            als = {}
            usbs = {}

            def _gather(key, out_ap, lo, it, slot):
                inst = nc.gpsimd.dma_gather(
                    out_ap=out_ap,
                    in_ap=tab_d[lo : lo + VOCAB, :],
                    idxs_ap=it[:, slot * IDXW_F : (slot + 1) * IDXW_F],
                    num_idxs=512,
                    num_idxs_reg=nreg,
                    elem_size=128,
                    transpose=True,
                    single_packet=True,
                    queue_num=queue_map.get(key, qcount[0] % NQ),
                )
                qcount[0] += 1
                gather_insts[inst.ins.name] = key

            def emit_gathers(blk):
                """39 single-field 512-idx gathers per block: even fields
                land directly in flat k-tile slots (with zeros on parts
                64:128), odd fields go to scratch then an SP-queue DMA
                merges them into the upper partitions."""
                it = idx_sb[blk]
                ft = flat_fm[blk]
                for g in range(KT):
                    f = min(2 * g, 38)
                    _gather((blk, g, 0),
                            ft[:, g * BLK : (g + 1) * BLK].rearrange(
                                "p (a n) -> p a n", a=1),
                            f * VOCAB, it, g)
                    if g < 19:
                        gtile = gatp.tile([128, BLK], dt.bfloat16, tag="g")
                        _gather((blk, g, 1),
                                gtile[:].rearrange("p (a n) -> p a n", a=1),
                                (2 * g + 1) * VOCAB, it, KT + g)
                        nc.sync.dma_start(
                            ft[64:128, g * BLK : (g + 1) * BLK],
                            gtile[0:64, 0:BLK])

            ztpss = {}

            def emit_ctrl_mms(blk, kts):
                """z^T = Wc^T @ flat k-tile accumulation (interleavable)."""
                ft = flat_fm[blk]
                if blk not in ztpss:
                    ztpss[blk] = pzt.tile([F, BLK], dt.float32, space="PSUM",
                                          tag="zt", name=f"ztps{blk}")
                ztps = ztpss[blk]
                for kt in kts:
                    nc.tensor.matmul(
                        ztps[:],
                        lhsT=wc_sb[:, kt * F : (kt + 1) * F],
                        rhs=ft[:, kt * BLK : (kt + 1) * BLK],
                        start=(kt == 0), stop=(kt == KT - 1),
                    )

            def emit_ctrl_out(blk):
                """DVE copy to SBUF adding the ctrl bias."""
                ztps = ztpss.pop(blk)
                zt_sb = ztp.tile([F, BLK], dt.float32, tag="z")
                nc.vector.tensor_scalar(zt_sb[:], ztps[:], cb_sb[:, 0:1],
                                        None, op0=OP.add)
                zts[blk] = zt_sb

            def emit_ztransp(blk):
                """z^T [F, BLK] -> batch-major z_all [128, 4*64] PSUM."""
                zt_sb = zts.pop(blk)
                z_all = pza.tile([128, 256], dt.float32, space="PSUM", tag="a")
                for c in range(NCHUNK):
                    nc.tensor.transpose(
                        out=z_all[:, c * 64 : c * 64 + F],
                        in_=zt_sb[:, c * 128 : (c + 1) * 128],
                        identity=idf[:F, :F])
                masks[blk] = z_all

            def emit_topk(blk):
                """Top-k + renormalized softmax -> batch-major mask (bf16)."""
                z_all = masks.pop(blk)
                for c in range(NCHUNK):
                    z = z_all[:, c * 64 : (c + 1) * 64]
                    mx = scr.tile([128, 8], dt.float32, tag="mx")
                    nm = scr.tile([128, 1], dt.float32, tag="nm")
                    zap = scr.tile([128, F], dt.float32, tag="zap")
                    zap2 = scr.tile([128, F], dt.float32, tag="zap2")
                    esb = scr.tile([128, F], dt.float32, tag="esb")
                    ssum = scr.tile([128, 1], dt.float32, tag="ssum")
                    rcp = scr.tile([128, 1], dt.float32, tag="rcp")
                    mbm = scr.tile([128, F], dt.bfloat16, tag="mbm")
                    src = z[:, :F]
                    outs = [zap[:], zap2[:]]
                    for r in range(rounds):
                        nc.vector.max(out=mx[:], in_=src)
                        if r == 0:
                            nc.vector.tensor_scalar(
                                nm[:], mx[:, 0:1], -1.0, None, op0=OP.mult)
                        if r == rounds - 1 and k - 8 * r < 8:
                            nc.vector.memset(mx[:, k - 8 * r :], -1e30)
                        nc.vector.match_replace(
                            out=outs[r % 2], in_to_replace=mx[:],
                            in_values=src, imm_value=-1e30)
                        src = outs[r % 2]
                    zfin = outs[(rounds - 1) % 2]
                    nc.scalar.activation(esb[:], z[:, :F], AF.Exp,
                                         bias=nm[:, 0:1], scale=1.0)
                    nc.vector.tensor_scalar(zfin, zfin, -1e30, None,
                                            op0=OP.is_equal)
                    nc.vector.tensor_tensor(esb[:], esb[:], zfin, op=OP.mult)
                    nc.vector.reduce_sum(ssum[:], esb[:],
                                         axis=mybir.AxisListType.X)
                    nc.vector.reciprocal(rcp[:], ssum[:])
                    nc.vector.tensor_scalar(mbm[:], esb[:], rcp[:, 0:1],
                                            None, op0=OP.mult)
                    masks[(blk, c)] = mbm

            def emit_masktransp(blk):
                """Batch-major masks -> feature-major mask_fm [F, BLK]."""
                mt_fm = mskp.tile([F, BLK], dt.bfloat16, tag="m")
                mt = pmt.tile([128, BLK], dt.bfloat16, space="PSUM", tag="t")
                for c in range(NCHUNK):
                    mbm = masks.pop((blk, c))
                    nc.tensor.transpose(
                        out=mt[:F, c * 128 : (c + 1) * 128], in_=mbm[:],
                        identity=idb[:])
                    nc.scalar.activation(
                        mt_fm[:, c * 128 : (c + 1) * 128],
                        mt[:F, c * 128 : (c + 1) * 128], AF.Copy, scale=1.0)
                mt_fms[blk] = mt_fm

            def emit_expand(blk):
                """Expand mask over features (PE, with SCALE_X baked into S)
                and multiply with raw flat -> fp8 x0 (DVE)."""
                ft = flat_fm[blk]
                mt_fm = mt_fms[blk]
                x0 = x0p.tile([128, KT * BLK], dt.float8e4, tag="x")
                for kt in range(KT):
                    ex = psb.tile([128, BLK], dt.float32, space="PSUM",
                                  tag="b")
                    nc.tensor.matmul(
                        ex[:], lhsT=s_sb[:, kt * 128 : (kt + 1) * 128],
                        rhs=mt_fm[:], start=True, stop=True)
                    nc.vector.tensor_tensor(
                        x0[:, kt * BLK : (kt + 1) * BLK],
                        ft[:, kt * BLK : (kt + 1) * BLK], ex[:],
                        op=OP.mult)
                x0s[blk] = x0

            def emit_u(blk):
                """U = x0 @ [cross_w.T | lin_w_a]: fp8 DR + Tu offset fix."""
                x0 = x0s[blk]
                mt_fm = mt_fms[blk]
                up = psb.tile([128, BLK], dt.float32, space="PSUM", tag="b")
                for G in range(GD):
                    nc.tensor.matmul(
                        up[:16, :],
                        lhsT=wu_sb[:, G * 32 : (G + 1) * 32].rearrange(
                            "p (s m) -> p s m", s=2),
                        rhs=x0[:, 2 * G * BLK : (2 * G + 2) * BLK].rearrange(
                            "p (s n) -> p s n", s=2),
                        start=(G == 0), stop=False,
                        perf_mode=mybir.MatmulPerfMode.DoubleRow,
                        skip_group_check=True,
                    )
                nc.tensor.matmul(
                    up[:7, :], lhsT=tu_sb[:], rhs=mt_fm[:],
                    start=False, stop=True, skip_group_check=True)
                u_sb = scr.tile([7, BLK], dt.float32, tag="usb", bufs=2)
                nc.vector.tensor_scalar(u_sb[:], up[:7, :], DESCALE, None,
                                        op0=OP.mult)
                usbs[blk] = u_sb

            def emit_mlp0(blk, ctrl_next=False):
                """MLP0: fp8 DR over 10 double-tiles + T0 offset fix, then
                fused BN+ReLU (with fp8 descale) on ACT.  ctrl(b+1) k-tile
                matmuls are sprinkled between m-tiles so the controller
                accumulates while its gathers land."""
                x0 = x0s.pop(blk)
                mt_fm = mt_fms.pop(blk)
                h0_fm = bigp.tile([128, M0 * BLK], dt.bfloat16, tag="h0")
                for m in range(M0):
                    if ctrl_next:
                        lo = (KT * m) // M0
                        hi = (KT * (m + 1)) // M0
                        emit_ctrl_mms(blk + 1, range(lo, hi))
                    hp = psb.tile([128, BLK], dt.float32, space="PSUM",
                                  tag="b")
                    for G in range(GD):
                        nc.tensor.matmul(
                            hp[:],
                            lhsT=w0_sb[:, (G * M0 + m) * 256 : (G * M0 + m + 1) * 256].rearrange(
                                "p (s m2) -> p s m2", s=2),
                            rhs=x0[:, 2 * G * BLK : (2 * G + 2) * BLK].rearrange(
                                "p (s n) -> p s n", s=2),
                            start=(G == 0), stop=False,
                            perf_mode=mybir.MatmulPerfMode.DoubleRow,
                            skip_group_check=True,
                        )
                    nc.tensor.matmul(
                        hp[:], lhsT=t0_sb[:, m * 128 : (m + 1) * 128],
                        rhs=mt_fm[:], start=False, stop=True,
                        skip_group_check=True)
                    nc.scalar.activation(h0_fm[:, m * BLK : (m + 1) * BLK],
                                         hp[:], AF.Relu,
                                         bias=b0_sb[:, m : m + 1],
                                         scale=DESCALE)
                h0s[blk] = h0_fm

            def emit_alpha(blk):
                """Transpose u to batch-major and run the cross-collapse
                scalar recursion on [128, 4] tiles (all 4 chunks at once)."""
                u_sb = usbs.pop(blk)
                ut_all = pss.tile([128, 256], dt.float32, space="PSUM",
                                  tag="s")
                for c in range(NCHUNK):
                    nc.tensor.transpose(
                        out=ut_all[:, c * 64 : c * 64 + 7],
                        in_=u_sb[:, c * 128 : (c + 1) * 128],
                        identity=idf[:7, :7],
                    )
                # level-major copy: usb4[:, 4l:4l+4] = u_l for the 4 chunks
                usb4 = scr.tile([128, 32], dt.float32, tag="u4")
                nc.vector.tensor_copy(
                    usb4[:, 0:28].rearrange("p (w c) -> p c w", c=4),
                    ut_all[:, 0:256].rearrange("p (c w) -> p c w", c=4)[:, :, 0:7],
                )
                al4 = scr.tile([128, 4], dt.float32, tag="al", bufs=2)
                t14 = scr.tile([128, 4], dt.float32, tag="t14")
                nc.vector.tensor_scalar(al4[:], usb4[:, 0:4],
                                        1.0 + v_consts[0], None, op0=OP.add)
                for l in range(1, L):
                    nc.vector.tensor_scalar(t14[:], usb4[:, 4 * l : 4 * l + 4],
                                            1.0, None, op0=OP.add)
                    nc.vector.tensor_tensor(al4[:], al4[:], t14[:],
                                            op=OP.mult)
                    if v_consts[l] != 0.0:
                        nc.vector.tensor_scalar(al4[:], al4[:],
                                                v_consts[l], None,
                                                op0=OP.add)
                nc.vector.tensor_tensor(al4[:], al4[:], usb4[:, 24:28],
                                        op=OP.mult)
                als[blk] = al4

            def emit_mlp1(blk):
                h0_fm = h0s.pop(blk)
                h1_fm = bigp.tile([128, M1 * BLK], dt.bfloat16, tag="h1")
                for m in range(M1):
                    hp = psb.tile([128, BLK], dt.float32, space="PSUM",
                                  tag="b")
                    for kt in range(KT1):
                        nc.tensor.matmul(
                            hp[:],
                            lhsT=w1_sb[:, (kt * M1 + m) * 128 : (kt * M1 + m + 1) * 128],
                            rhs=h0_fm[:, kt * BLK : (kt + 1) * BLK],
                            start=(kt == 0), stop=(kt == KT1 - 1),
                        )
                    nc.scalar.activation(h1_fm[:, m * BLK : (m + 1) * BLK],
                                         hp[:], AF.Relu,
                                         bias=b1_sb[:, m : m + 1], scale=1.0)
                h1s[blk] = h1_fm

            def emit_final(blk):
                """r = h1 . lin_w_b ; p = sigmoid(alpha + r + c0)."""
                h1_fm = h1s.pop(blk)
                al4 = als.pop(blk)
                rp_all = pss.tile([128, 256], dt.float32, space="PSUM",
                                  tag="s")
                for c in range(NCHUNK):
                    for kt in range(M1):
                        nc.tensor.matmul(
                            rp_all[:, c * 64 : c * 64 + 1],
                            lhsT=h1_fm[:, kt * BLK + c * 128 : kt * BLK + (c + 1) * 128],
                            rhs=lw2_sb[:, kt : kt + 1],
                            start=(kt == 0), stop=(kt == M1 - 1),
                        )
                    t2 = scr.tile([128, 1], dt.float32, tag="t2")
                    nc.vector.tensor_tensor(t2[:], al4[:, c : c + 1],
                                            rp_all[:, c * 64 : c * 64 + 1],
                                            op=OP.add)
                    nc.scalar.activation(
                        p_sb[:, blk * NCHUNK + c : blk * NCHUNK + c + 1],
                        t2[:], AF.Sigmoid, bias=float(c0), scale=1.0)

            # ================= schedule =================
            for blk in range(NBLK):
                emit_gathers(blk)

            # prologue: block 0 control path trickles under its gathers
            emit_ctrl_mms(0, range(KT))
            emit_ctrl_out(0)
            emit_ztransp(0)
            emit_topk(0)
            emit_masktransp(0)

            for blk in range(NBLK):
                if blk >= 1:
                    emit_mlp1(blk - 1)
                    emit_final(blk - 1)
                emit_expand(blk)
                emit_u(blk)
                emit_mlp0(blk, ctrl_next=(blk + 1 < NBLK))
                emit_alpha(blk)
                if blk + 1 < NBLK:
                    emit_ctrl_out(blk + 1)
                    emit_ztransp(blk + 1)
                    emit_topk(blk + 1)
                    emit_masktransp(blk + 1)
            emit_mlp1(NBLK - 1)
            emit_final(NBLK - 1)

            # ---- transpose p [128, 16] -> [16, 128] and store ----
            ptp = pss.tile([128, 256], dt.float32, space="PSUM", tag="s")
            nc.tensor.transpose(out=ptp[: BPC // 128, :128], in_=p_sb[:],
                                identity=idf[:])
            pout = cpool.tile([BPC // 128, 128], dt.float32)
            nc.vector.tensor_copy(pout[:], ptp[: BPC // 128, :128])
            nc.sync.dma_start(out_d[:].rearrange("(a b) -> a b", b=128),
                              pout[:])

    nc.compile()
    return nc


def _prep_host(inputs):
    """Host-side preprocessing -> per-core input maps."""
    x = np.asarray(inputs["x"]).astype(np.int64)
    tab = np.asarray(inputs["emb_table"], dtype=np.float32)
    k = int(np.asarray(inputs["k"]))

    s_f = (np.asarray(inputs["bn_gamma"], np.float64)
           / np.sqrt(np.asarray(inputs["bn_var"], np.float64) + EPS))
    t_f = np.asarray(inputs["bn_beta"], np.float64) - np.asarray(
        inputs["bn_mean"], np.float64) * s_f
    # table carries only the BN scale; offsets go through T0/Tu/c_ctrl
    tab_raw = tab.astype(np.float64) * np.repeat(s_f, VOCAB)[:, None]
    tab_h = np.zeros((F * VOCAB, 128), bf16)
    tab_h[:, :E] = tab_raw.astype(bf16)

    ctrl_w = np.asarray(inputs["ctrl_w"], np.float64)  # [D, F]
    wc = np.zeros((KT * 128, F), np.float32)
    wc[:D] = ctrl_w.astype(np.float32)
    wc_h = np.ascontiguousarray(
        wc.reshape(KT, 128, F).transpose(1, 0, 2).reshape(128, KT * F)).astype(bf16)
    # ctrl bias + exact offset contribution: c_ctrl[j] = sum_f t_f*sum_e Wc
    c_ctrl = (t_f[:, None] * ctrl_w.reshape(F, E, F).sum(axis=1)).sum(axis=0)
    cb_h = (np.asarray(inputs["ctrl_b"], np.float64) + c_ctrl).astype(
        np.float32).reshape(F, 1)

    # MLP0 with BN scale folded into columns; fp8 DoubleRow plane-major pack
    g0 = (np.asarray(inputs["mlp_g0"], np.float64)
          / np.sqrt(np.asarray(inputs["mlp_v0"], np.float64) + EPS))
    w0g = np.asarray(inputs["mlp_w0"], np.float64) * g0[None, :]  # [D, H0]
    w0p = np.zeros((KT * 128, H0), np.float32)
    w0p[:D] = (w0g * SCALE_W).astype(np.float32)
    w0_8 = w0p.astype(f8)  # [2560, 1024]
    # layout [128, GD*M0*256]: per (G, m): [sub0 plane 128 | sub1 plane 128]
    w0_h = np.zeros((128, GD * M0 * 256), f8)
    for G in range(GD):
        for m in range(M0):
            base = (G * M0 + m) * 256
            for sub in range(2):
                w0_h[:, base + sub * 128 : base + (sub + 1) * 128] = \
                    w0_8[(2 * G + sub) * 128 : (2 * G + sub + 1) * 128,
                         m * 128 : (m + 1) * 128]
    # offset correction: T0[f, j] = t_f * sum_e w0g[f*64+e, j], pre-scaled
    t0_h = ((t_f[:, None] * w0g.reshape(F, E, H0).sum(axis=1))
            * (SCALE_X * SCALE_W)).astype(bf16)
    b0 = ((np.asarray(inputs["mlp_b0"], np.float64)
           - np.asarray(inputs["mlp_m0"], np.float64)) * g0
          + np.asarray(inputs["mlp_be0"], np.float64)).astype(np.float32)
    b0_h = np.ascontiguousarray(b0.reshape(M0, 128).T)

    g1 = (np.asarray(inputs["mlp_g1"], np.float64)
          / np.sqrt(np.asarray(inputs["mlp_v1"], np.float64) + EPS))
    w1 = np.asarray(inputs["mlp_w1"], np.float32) * g1[None, :].astype(np.float32)
    b1 = ((np.asarray(inputs["mlp_b1"], np.float64)
           - np.asarray(inputs["mlp_m1"], np.float64)) * g1
          + np.asarray(inputs["mlp_be1"], np.float64)).astype(np.float32)
    w1_h = np.ascontiguousarray(
        w1.reshape(KT1, 128, M1, 128).transpose(1, 0, 2, 3)
        .reshape(128, KT1 * M1 * 128)).astype(bf16)
    b1_h = np.ascontiguousarray(b1.reshape(M1, 128).T)

    # U weights: 6 cross rows + lin_w[:D]; fp8 DR plane-major [128, GD*14]
    cross_w = np.asarray(inputs["cross_w"], np.float64)
    cross_b = np.asarray(inputs["cross_b"], np.float64)
    lin_w = np.asarray(inputs["lin_w"], np.float64)
    wu = np.zeros((KT * 128, 7), np.float64)
    wu[:D, :L] = cross_w.T
    wu[:D, 6] = lin_w[:D]
    wu_8 = (wu * SCALE_W).astype(np.float32).astype(f8)
    wu_h = np.zeros((128, GD * 32), f8)
    for G in range(GD):
        for sub in range(2):
            wu_h[:, G * 32 + sub * 16 : G * 32 + sub * 16 + 7] = \
                wu_8[(2 * G + sub) * 128 : (2 * G + sub + 1) * 128, :]
    tu_h = ((t_f[:, None] * wu[:D].reshape(F, E, 7).sum(axis=1))
            * (SCALE_X * SCALE_W)).astype(bf16)

    # expand matrix S [F, KT*128] with the fp8 activation scale baked in
    s = np.zeros((F, KT * 128), np.float32)
    feat = np.arange(KT * 128)
    valid = feat < D
    s[feat[valid] // E, feat[valid]] = SCALE_X
    s_h = s.astype(bf16)

    lw2_h = np.ascontiguousarray(
        lin_w[D:].astype(np.float32).reshape(M1, 128).T).astype(bf16)

    # cross-collapse constants: v_l = beta_l . w_l ; c0 = beta_6 . lin_w_a + b
    beta = np.zeros(D, np.float64)
    v = np.zeros(L, np.float64)
    for l in range(L):
        v[l] = beta @ cross_w[l]
        beta = beta + cross_b[l]
    c0 = float(beta @ lin_w[:D]
               + float(np.asarray(inputs["lin_b"]).ravel()[0]))
    v_consts = tuple(float(t) for t in v)

    def wrap16(jj):
        # idx j at [j % 16, j // 16], replicated over 8 partition groups
        w = jj.reshape(-1, 16).T.astype(np.int16)  # [16, n/16]
        return np.tile(w, (8, 1))  # [128, n/16]

    in_maps = []
    for ci in range(NCORES):
        xs = x[ci * BPC : (ci + 1) * BPC]  # [2048, 39]
        idxs = np.zeros((NBLK, 128, IDXW_TOT), np.int16)
        for blk in range(NBLK):
            rows = xs[blk * BLK : (blk + 1) * BLK]  # [512, 39]
            for g in range(KT):  # even fields (direct), slot g
                f = min(2 * g, 38)
                idxs[blk, :, g * IDXW_F : (g + 1) * IDXW_F] = \
                    wrap16(rows[:, f].copy())
            for g in range(19):  # odd fields (scratch), slot 20+g
                idxs[blk, :, (KT + g) * IDXW_F : (KT + g + 1) * IDXW_F] = \
                    wrap16(rows[:, 2 * g + 1].copy())
        in_maps.append({
            "idxs": idxs,
            "tab": tab_h,
            "wc": wc_h,
            "cb": cb_h,
            "w0": w0_h,
            "t0": t0_h,
            "w1": w1_h,
            "wu": wu_h,
            "tu": tu_h,
            "s": s_h,
            "lw2": lw2_h,
            "b0": b0_h,
            "b1": b1_h,
        })
    return in_maps, k, v_consts, c0


def _scheduled_gather_queues(nc):
    """Walk the scheduled program; return [(key, ordinal, queue)] for
    every dma_gather, in scheduled (program) order."""
    out = []
    cnt = 0
    for f in nc.m.functions:
        for bb in f.blocks:
            for inst in bb.instructions:
                if type(inst).__name__ == "InstDMAGatherAnt":
                    key = nc._gather_insts.get(inst.name)
                    out.append((key, cnt, inst.queue_num))
                    cnt += 1
    return out


def _build_aligned(k, v_consts, c0):
    """Build, then verify the SWDGE queue assignment is consistent with
    the scheduler's mod-8 DMASW semaphore rotation (sem lane = scheduled
    ordinal % 8, each lane locked to one queue). If not, rebuild with
    queue = scheduled ordinal % NQ (fixpoint, few iterations)."""
    queue_map = {}
    for attempt in range(4):
        nc = _build(k, v_consts, c0, queue_map)
        sched = _scheduled_gather_queues(nc)
        lane_lock = {}
        ok = True
        for key, ordinal, q in sched:
            lane = ordinal % 8
            if lane_lock.setdefault(lane, q) != q:
                ok = False
        if ok:
            return nc
        new_map = {key: ordinal % NQ for key, ordinal, q in sched
                   if key is not None}
        if new_map == queue_map:
            return nc  # schedule oscillates; give up realigning
        queue_map = new_map
    return nc


def _get_nc(k, v_consts, c0):
    key = (k, v_consts, c0)
    if key not in _CACHE:
        _CACHE[key] = _build_aligned(k, v_consts, c0)
    return _CACHE[key]


def kernel(**inputs) -> np.ndarray:
    in_maps, k, v_consts, c0 = _prep_host(inputs)
    nc = _get_nc(k, v_consts, c0)
    res = run_bass_kernel_spmd(nc, in_maps, core_ids=list(range(NCORES)))
    out = np.concatenate([res.results[i]["out"] for i in range(NCORES)])
    return out.astype(np.float32)


def run_traced(**inputs):
    """Like kernel() but with tracing enabled; returns (out, results)."""
    in_maps, k, v_consts, c0 = _prep_host(inputs)
    nc = _get_nc(k, v_consts, c0)
    res = run_bass_kernel_spmd(nc, in_maps, core_ids=list(range(NCORES)),
                               trace=True)
    out = np.concatenate([res.results[i]["out"] for i in range(NCORES)])
    return out.astype(np.float32), res


# revision 7
# speedup vs baseline: 1.0094x; 1.0094x over previous
"""Trainium2 Bass kernel for nn_DeepCrossNetworkModel_Controller_hard.

Model: per-field embedding gather -> BatchNorm1d(F) (eval) -> controller
linear + softmax over fields -> top-k mask (renormalized) -> CrossNetwork(6)
+ MLP(2496->1024->512, BN+ReLU) -> concat -> linear -> sigmoid.

Strategy (data-parallel over 8 NeuronCores, 2048 rows each):
 - BN *scale* folded into the embedding table on host (bf16, rows padded to
   128 so dma_gather(transpose=True) lands embeddings feature-major).  The
   BN *offset* (beta - mean*scale, constant per field) is carried exactly
   through rank-39 correction matmuls (T0/Tu/c_ctrl), which lets the bulk
   MLP0/U matmuls run in fp8 DoubleRow (2 contraction planes per pass)
   without precision loss: fp8 only sees the small varying part.
 - All 156 gathers (39 fields x 4 blocks, 512 idxs each - the HW cap) are
   issued upfront; they serialize on the GPSIMD engine (~1.3us each), so
   the gather stream paces the pipeline and everything else hides under it.
 - Controller computed as z^T = Wc^T @ flat (20 N=512 matmuls/block,
   accumulated in PSUM as gathers land) + PE transpose to batch-major.
 - top-k of softmax + renormalize == softmax restricted to top-k logits.
 - CrossNetwork collapses algebraically: on device only U = x0 @ wu
   (7 cols) + a scalar recursion on [128,4] tiles.
"""

import sys

if "/opt/trn_rl_repo" not in sys.path:
    sys.path.insert(0, "/opt/trn_rl_repo")

import ml_dtypes
import numpy as np

import concourse.bass as bass
import concourse.bacc as bacc
import concourse.mybir as mybir
import concourse.tile as tile
from concourse.bass_utils import run_bass_kernel_spmd
from concourse.masks import make_identity

# Problem constants (hardcoded per spec).
B, F, E, L = 16384, 39, 64, 6
VOCAB = 10000
D = F * E  # 2496
H0, H1 = 1024, 512
EPS = 1e-5
NCORES = 8
BPC = B // NCORES      # 2048 rows per core
BLK = 512              # batch block
NBLK = BPC // BLK      # 4
NCHUNK = BLK // 128    # 4 chunks of 128 rows per block
KT = 20                # feature k-tiles of 128 (D padded 2496 -> 2560)
GD = KT // 2           # 10 fp8 DoubleRow double-tiles (256 contraction)
M0 = H0 // 128         # 8
M1 = H1 // 128         # 4
KT1 = H0 // 128        # 8
NQ = 4                 # SWDGE queues
SCALE_X = 256.0        # fp8 activation scale (baked into expand matrix S)
SCALE_W = 64.0         # fp8 weight scale
DESCALE = 1.0 / (SCALE_X * SCALE_W)   # 2^-14
# idx widths (wrapped-by-16 free dims): 39 single-field gathers of 512
# (transpose-mode dma_gather hangs on HW above 512 idxs per call)
IDXW_F = 512 // 16       # 32
IDXW_TOT = 39 * IDXW_F   # 1248: 20 even-field (direct) + 19 odd (scratch)

dt = mybir.dt
AF = mybir.ActivationFunctionType
OP = mybir.AluOpType
bf16 = ml_dtypes.bfloat16
f8 = ml_dtypes.float8_e4m3

_CACHE = {}


def _build(k, v_consts, c0, queue_map=None):
    """Build the per-core SPMD bass module.

    queue_map: optional {(blk, g, j): queue_num} overriding the default
    round-robin assignment, used to realign SWDGE queues with the
    scheduler's mod-8 DMASW semaphore rotation (see _build_aligned).
    """
    queue_map = queue_map or {}
    gather_insts = {}
    nc = bacc.Bacc("TRN2", target_bir_lowering=False, debug=False,
                   num_devices=NCORES, num_swdge_queues=NQ)
    nc._gather_insts = gather_insts

    idxs_d = nc.declare_dram_parameter("idxs", [NBLK, 128, IDXW_TOT], dt.int16, isOutput=False)
    tab_d = nc.declare_dram_parameter("tab", [F * VOCAB, 128], dt.bfloat16, isOutput=False)
    wc_d = nc.declare_dram_parameter("wc", [128, KT * F], dt.bfloat16, isOutput=False)
    cb_d = nc.declare_dram_parameter("cb", [F, 1], dt.float32, isOutput=False)
    w0_d = nc.declare_dram_parameter("w0", [128, GD * M0 * 256], dt.float8e4, isOutput=False)
    t0_d = nc.declare_dram_parameter("t0", [F, H0], dt.bfloat16, isOutput=False)
    w1_d = nc.declare_dram_parameter("w1", [128, KT1 * M1 * 128], dt.bfloat16, isOutput=False)
    wu_d = nc.declare_dram_parameter("wu", [128, GD * 32], dt.float8e4, isOutput=False)
    tu_d = nc.declare_dram_parameter("tu", [F, 7], dt.bfloat16, isOutput=False)
    s_d = nc.declare_dram_parameter("s", [F, KT * 128], dt.bfloat16, isOutput=False)
    lw2_d = nc.declare_dram_parameter("lw2", [128, M1], dt.bfloat16, isOutput=False)
    b0_d = nc.declare_dram_parameter("b0", [128, M0], dt.float32, isOutput=False)
    b1_d = nc.declare_dram_parameter("b1", [128, M1], dt.float32, isOutput=False)
    out_d = nc.declare_dram_parameter("out", [BPC], dt.float32, isOutput=True)

    rounds = -(-k // 8)  # ceil(k/8) match_replace rounds

    with tile.TileContext(nc) as tc:
        with (
            tc.tile_pool(name="const", bufs=1) as cpool,
            tc.tile_pool(name="flat", bufs=1) as flatp,
            tc.tile_pool(name="x0", bufs=2) as x0p,
            tc.tile_pool(name="big", bufs=2) as bigp,
            tc.tile_pool(name="msk", bufs=2) as mskp,
            tc.tile_pool(name="zt", bufs=2) as ztp,
            tc.tile_pool(name="gat", bufs=12) as gatp,
            tc.tile_pool(name="scr", bufs=8) as scr,
            # PSUM slots are bank-granular (8 banks x 2KB/partition):
            # psb x3 (ex/up/hp) + pzt x1 + pza x1 + pmt x1 + pss x2
            tc.tile_pool(name="psb", bufs=3, space="PSUM") as psb,
            tc.tile_pool(name="pzt", bufs=1, space="PSUM") as pzt,
            tc.tile_pool(name="pza", bufs=1, space="PSUM") as pza,
            tc.tile_pool(name="pmt", bufs=1, space="PSUM") as pmt,
            tc.tile_pool(name="pss", bufs=2, space="PSUM") as pss,
        ):
            # ---- identities + idx DMAs first (cheap, unblock gathers) ----
            idf = cpool.tile([128, 128], dt.float32)
            make_identity(nc, idf[:])
            idb = cpool.tile([128, 128], dt.bfloat16)
            make_identity(nc, idb[:])

            idx_sb = []
            for b in range(NBLK):
                t = cpool.tile([128, IDXW_TOT], dt.int16, name=f"idx{b}")
                nc.sync.dma_start(t[:], idxs_d[b, :, :])
                idx_sb.append(t)

            # ---- weights on the scalar HWDGE queue (small consts first) ----
            wc_sb = cpool.tile([128, KT * F], dt.bfloat16)
            nc.scalar.dma_start(wc_sb[:], wc_d[:])
            cb_sb = cpool.tile([F, 1], dt.float32)
            nc.scalar.dma_start(cb_sb[:], cb_d[:])
            s_sb = cpool.tile([F, KT * 128], dt.bfloat16)
            nc.scalar.dma_start(s_sb[:], s_d[:])
            wu_sb = cpool.tile([128, GD * 32], dt.float8e4)
            nc.scalar.dma_start(wu_sb[:], wu_d[:])
            tu_sb = cpool.tile([F, 7], dt.bfloat16)
            nc.scalar.dma_start(tu_sb[:], tu_d[:])
            t0_sb = cpool.tile([F, H0], dt.bfloat16)
            nc.scalar.dma_start(t0_sb[:], t0_d[:])
            lw2_sb = cpool.tile([128, M1], dt.bfloat16)
            nc.scalar.dma_start(lw2_sb[:], lw2_d[:])
            b0_sb = cpool.tile([128, M0], dt.float32)
            nc.scalar.dma_start(b0_sb[:], b0_d[:])
            b1_sb = cpool.tile([128, M1], dt.float32)
            nc.scalar.dma_start(b1_sb[:], b1_d[:])
            w0_sb = cpool.tile([128, GD * M0 * 256], dt.float8e4)
            for q in range(4):
                qs = GD * M0 * 256 // 4
                nc.scalar.dma_start(w0_sb[:, q * qs : (q + 1) * qs],
                                    w0_d[:, q * qs : (q + 1) * qs])
            w1_sb = cpool.tile([128, KT1 * M1 * 128], dt.bfloat16)
            nc.scalar.dma_start(w1_sb[:], w1_d[:])

            # ---- persistent activations ----
            flat_fm = [flatp.tile([128, KT * BLK], dt.bfloat16, tag=f"f{i}",
                                  name=f"flat{i}")
                       for i in range(NBLK)]
            p_sb = cpool.tile([128, BPC // 128], dt.float32)

            nreg = nc.gpsimd.to_reg(512)
            qcount = [0]  # global SWDGE queue round-robin

            gts = {}
            masks = {}
            mt_fms = {}
            zts = {}
            x0s = {}
            h0s = {}
            h1s = {}
            als = {}
            usbs = {}

            def _gather(key, out_ap, lo, it, slot):
                inst = nc.gpsimd.dma_gather(
                    out_ap=out_ap,
                    in_ap=tab_d[lo : lo + VOCAB, :],
                    idxs_ap=it[:, slot * IDXW_F : (slot + 1) * IDXW_F],
                    num_idxs=512,
                    num_idxs_reg=nreg,
                    elem_size=128,
                    transpose=True,
                    single_packet=True,
                    queue_num=queue_map.get(key, qcount[0] % NQ),
                )
                qcount[0] += 1
                gather_insts[inst.ins.name] = key

            def emit_gathers(blk):
                """39 single-field 512-idx gathers per block: even fields
                land directly in flat k-tile slots (with zeros on parts
                64:128), odd fields go to scratch then an SP-queue DMA
                merges them into the upper partitions."""
                it = idx_sb[blk]
                ft = flat_fm[blk]
                for g in range(KT):
                    f = min(2 * g, 38)
                    _gather((blk, g, 0),
                            ft[:, g * BLK : (g + 1) * BLK].rearrange(
                                "p (a n) -> p a n", a=1),
                            f * VOCAB, it, g)
                    if g < 19:
                        gtile = gatp.tile([128, BLK], dt.bfloat16, tag="g")
                        _gather((blk, g, 1),
                                gtile[:].rearrange("p (a n) -> p a n", a=1),
                                (2 * g + 1) * VOCAB, it, KT + g)
                        nc.sync.dma_start(
                            ft[64:128, g * BLK : (g + 1) * BLK],
                            gtile[0:64, 0:BLK])

            ztpss = {}

            def emit_ctrl_mms(blk, kts):
                """z^T = Wc^T @ flat k-tile accumulation (interleavable)."""
                ft = flat_fm[blk]
                if blk not in ztpss:
                    ztpss[blk] = pzt.tile([F, BLK], dt.float32, space="PSUM",
                                          tag="zt", name=f"ztps{blk}")
                ztps = ztpss[blk]
                for kt in kts:
                    nc.tensor.matmul(
                        ztps[:],
                        lhsT=wc_sb[:, kt * F : (kt + 1) * F],
                        rhs=ft[:, kt * BLK : (kt + 1) * BLK],
                        start=(kt == 0), stop=(kt == KT - 1),
                    )

            def emit_ctrl_out(blk):
                """DVE copy to SBUF adding the ctrl bias."""
                ztps = ztpss.pop(blk)
                zt_sb = ztp.tile([F, BLK], dt.float32, tag="z")
                nc.vector.tensor_scalar(zt_sb[:], ztps[:], cb_sb[:, 0:1],
                                        None, op0=OP.add)
                zts[blk] = zt_sb

            def emit_ztransp(blk):
                """z^T [F, BLK] -> batch-major z_all [128, 4*64] PSUM."""
                zt_sb = zts.pop(blk)
                z_all = pza.tile([128, 256], dt.float32, space="PSUM", tag="a")
                for c in range(NCHUNK):
                    nc.tensor.transpose(
                        out=z_all[:, c * 64 : c * 64 + F],
                        in_=zt_sb[:, c * 128 : (c + 1) * 128],
                        identity=idf[:F, :F])
                masks[blk] = z_all

            def emit_topk(blk):
                """Top-k + renormalized softmax -> batch-major mask (bf16)."""
                z_all = masks.pop(blk)
                for c in range(NCHUNK):
                    z = z_all[:, c * 64 : (c + 1) * 64]
                    mx = scr.tile([128, 8], dt.float32, tag="mx")
                    nm = scr.tile([128, 1], dt.float32, tag="nm")
                    zap = scr.tile([128, F], dt.float32, tag="zap")
                    zap2 = scr.tile([128, F], dt.float32, tag="zap2")
                    esb = scr.tile([128, F], dt.float32, tag="esb")
                    ssum = scr.tile([128, 1], dt.float32, tag="ssum")
                    rcp = scr.tile([128, 1], dt.float32, tag="rcp")
                    mbm = scr.tile([128, F], dt.bfloat16, tag="mbm")
                    src = z[:, :F]
                    outs = [zap[:], zap2[:]]
                    for r in range(rounds):
                        nc.vector.max(out=mx[:], in_=src)
                        if r == 0:
                            nc.vector.tensor_scalar(
                                nm[:], mx[:, 0:1], -1.0, None, op0=OP.mult)
                        if r == rounds - 1 and k - 8 * r < 8:
                            nc.vector.memset(mx[:, k - 8 * r :], -1e30)
                        nc.vector.match_replace(
                            out=outs[r % 2], in_to_replace=mx[:],
                            in_values=src, imm_value=-1e30)
                        src = outs[r % 2]
                    zfin = outs[(rounds - 1) % 2]
                    nc.scalar.activation(esb[:], z[:, :F], AF.Exp,
                                         bias=nm[:, 0:1], scale=1.0)
                    nc.vector.tensor_scalar(zfin, zfin, -1e30, None,
                                            op0=OP.is_equal)
                    nc.vector.tensor_tensor(esb[:], esb[:], zfin, op=OP.mult)
                    nc.vector.reduce_sum(ssum[:], esb[:],
                                         axis=mybir.AxisListType.X)
                    nc.vector.reciprocal(rcp[:], ssum[:])
                    nc.vector.tensor_scalar(mbm[:], esb[:], rcp[:, 0:1],
                                            None, op0=OP.mult)
                    masks[(blk, c)] = mbm

            def emit_masktransp(blk):
                """Batch-major masks -> feature-major mask_fm [F, BLK]."""
                mt_fm = mskp.tile([F, BLK], dt.bfloat16, tag="m")
                mt = pmt.tile([128, BLK], dt.bfloat16, space="PSUM", tag="t")
                for c in range(NCHUNK):
                    mbm = masks.pop((blk, c))
                    nc.tensor.transpose(
                        out=mt[:F, c * 128 : (c + 1) * 128], in_=mbm[:],
                        identity=idb[:])
                    nc.scalar.activation(
                        mt_fm[:, c * 128 : (c + 1) * 128],
                        mt[:F, c * 128 : (c + 1) * 128], AF.Copy, scale=1.0)
                mt_fms[blk] = mt_fm

            def emit_expand(blk):
                """Expand mask over features (PE, with SCALE_X baked into S)
                and multiply with raw flat -> fp8 x0 (DVE)."""
                ft = flat_fm[blk]
                mt_fm = mt_fms[blk]
                x0 = x0p.tile([128, KT * BLK], dt.float8e4, tag="x")
                for kt in range(KT):
                    ex = psb.tile([128, BLK], dt.float32, space="PSUM",
                                  tag="b")
                    nc.tensor.matmul(
                        ex[:], lhsT=s_sb[:, kt * 128 : (kt + 1) * 128],
                        rhs=mt_fm[:], start=True, stop=True)
                    nc.vector.tensor_tensor(
                        x0[:, kt * BLK : (kt + 1) * BLK],
                        ft[:, kt * BLK : (kt + 1) * BLK], ex[:],
                        op=OP.mult)
                x0s[blk] = x0

            def emit_u(blk):
                """U = x0 @ [cross_w.T | lin_w_a]: fp8 DR + Tu offset fix."""
                x0 = x0s[blk]
                mt_fm = mt_fms[blk]
                up = psb.tile([128, BLK], dt.float32, space="PSUM", tag="b")
                for G in range(GD):
                    nc.tensor.matmul(
                        up[:16, :],
                        lhsT=wu_sb[:, G * 32 : (G + 1) * 32].rearrange(
                            "p (s m) -> p s m", s=2),
                        rhs=x0[:, 2 * G * BLK : (2 * G + 2) * BLK].rearrange(
                            "p (s n) -> p s n", s=2),
                        start=(G == 0), stop=False,
                        perf_mode=mybir.MatmulPerfMode.DoubleRow,
                        skip_group_check=True,
                    )
                nc.tensor.matmul(
                    up[:7, :], lhsT=tu_sb[:], rhs=mt_fm[:],
                    start=False, stop=True, skip_group_check=True)
                u_sb = scr.tile([7, BLK], dt.float32, tag="usb", bufs=2)
                nc.vector.tensor_scalar(u_sb[:], up[:7, :], DESCALE, None,
                                        op0=OP.mult)
                usbs[blk] = u_sb

            def emit_mlp0(blk):
                """MLP0: fp8 DR over 10 double-tiles + T0 offset fix, then
                fused BN+ReLU (with fp8 descale) on ACT."""
                x0 = x0s.pop(blk)
                mt_fm = mt_fms.pop(blk)
                h0_fm = bigp.tile([128, M0 * BLK], dt.bfloat16, tag="h0")
                for m in range(M0):
                    hp = psb.tile([128, BLK], dt.float32, space="PSUM",
                                  tag="b")
                    for G in range(GD):
                        nc.tensor.matmul(
                            hp[:],
                            lhsT=w0_sb[:, (G * M0 + m) * 256 : (G * M0 + m + 1) * 256].rearrange(
                                "p (s m2) -> p s m2", s=2),
                            rhs=x0[:, 2 * G * BLK : (2 * G + 2) * BLK].rearrange(
                                "p (s n) -> p s n", s=2),
                            start=(G == 0), stop=False,
                            perf_mode=mybir.MatmulPerfMode.DoubleRow,
                            skip_group_check=True,
                        )
                    nc.tensor.matmul(
                        hp[:], lhsT=t0_sb[:, m * 128 : (m + 1) * 128],
                        rhs=mt_fm[:], start=False, stop=True,
                        skip_group_check=True)
                    nc.scalar.activation(h0_fm[:, m * BLK : (m + 1) * BLK],
                                         hp[:], AF.Relu,
                                         bias=b0_sb[:, m : m + 1],
                                         scale=DESCALE)
                h0s[blk] = h0_fm

            def emit_alpha(blk):
                """Transpose u to batch-major and run the cross-collapse
                scalar recursion on [128, 4] tiles (all 4 chunks at once)."""
                u_sb = usbs.pop(blk)
                ut_all = pss.tile([128, 256], dt.float32, space="PSUM",
                                  tag="s")
                for c in range(NCHUNK):
                    nc.tensor.transpose(
                        out=ut_all[:, c * 64 : c * 64 + 7],
                        in_=u_sb[:, c * 128 : (c + 1) * 128],
                        identity=idf[:7, :7],
                    )
                # level-major copy: usb4[:, 4l:4l+4] = u_l for the 4 chunks
                usb4 = scr.tile([128, 32], dt.float32, tag="u4")
                nc.vector.tensor_copy(
                    usb4[:, 0:28].rearrange("p (w c) -> p c w", c=4),
                    ut_all[:, 0:256].rearrange("p (c w) -> p c w", c=4)[:, :, 0:7],
                )
                al4 = scr.tile([128, 4], dt.float32, tag="al", bufs=2)
                t14 = scr.tile([128, 4], dt.float32, tag="t14")
                nc.vector.tensor_scalar(al4[:], usb4[:, 0:4],
                                        1.0 + v_consts[0], None, op0=OP.add)
                for l in range(1, L):
                    nc.vector.tensor_scalar(t14[:], usb4[:, 4 * l : 4 * l + 4],
                                            1.0, None, op0=OP.add)
                    nc.vector.tensor_tensor(al4[:], al4[:], t14[:],
                                            op=OP.mult)
                    if v_consts[l] != 0.0:
                        nc.vector.tensor_scalar(al4[:], al4[:],
                                                v_consts[l], None,
                                                op0=OP.add)
                nc.vector.tensor_tensor(al4[:], al4[:], usb4[:, 24:28],
                                        op=OP.mult)
                als[blk] = al4

            def emit_mlp1(blk):
                h0_fm = h0s.pop(blk)
                h1_fm = bigp.tile([128, M1 * BLK], dt.bfloat16, tag="h1")
                for m in range(M1):
                    hp = psb.tile([128, BLK], dt.float32, space="PSUM",
                                  tag="b")
                    for kt in range(KT1):
                        nc.tensor.matmul(
                            hp[:],
                            lhsT=w1_sb[:, (kt * M1 + m) * 128 : (kt * M1 + m + 1) * 128],
                            rhs=h0_fm[:, kt * BLK : (kt + 1) * BLK],
                            start=(kt == 0), stop=(kt == KT1 - 1),
                        )
                    nc.scalar.activation(h1_fm[:, m * BLK : (m + 1) * BLK],
                                         hp[:], AF.Relu,
                                         bias=b1_sb[:, m : m + 1], scale=1.0)
                h1s[blk] = h1_fm

            def emit_final(blk):
                """r = h1 . lin_w_b ; p = sigmoid(alpha + r + c0)."""
                h1_fm = h1s.pop(blk)
                al4 = als.pop(blk)
                rp_all = pss.tile([128, 256], dt.float32, space="PSUM",
                                  tag="s")
                for c in range(NCHUNK):
                    for kt in range(M1):
                        nc.tensor.matmul(
                            rp_all[:, c * 64 : c * 64 + 1],
                            lhsT=h1_fm[:, kt * BLK + c * 128 : kt * BLK + (c + 1) * 128],
                            rhs=lw2_sb[:, kt : kt + 1],
                            start=(kt == 0), stop=(kt == M1 - 1),
                        )
                    t2 = scr.tile([128, 1], dt.float32, tag="t2")
                    nc.vector.tensor_tensor(t2[:], al4[:, c : c + 1],
                                            rp_all[:, c * 64 : c * 64 + 1],
                                            op=OP.add)
                    nc.scalar.activation(
                        p_sb[:, blk * NCHUNK + c : blk * NCHUNK + c + 1],
                        t2[:], AF.Sigmoid, bias=float(c0), scale=1.0)

            # ================= schedule =================
            for blk in range(NBLK):
                emit_gathers(blk)

            # prologue: block 0 control path trickles under its gathers
            emit_ctrl_mms(0, range(KT))
            emit_ctrl_out(0)
            emit_ztransp(0)
            emit_topk(0)
            emit_masktransp(0)

            for blk in range(NBLK):
                if blk >= 1:
                    emit_mlp1(blk - 1)
                    emit_final(blk - 1)
                emit_expand(blk)
                emit_u(blk)
                emit_mlp0(blk)
                emit_alpha(blk)
                if blk + 1 < NBLK:
                    emit_ctrl_mms(blk + 1, range(KT))
                    emit_ctrl_out(blk + 1)
                    emit_ztransp(blk + 1)
                    emit_topk(blk + 1)
                    emit_masktransp(blk + 1)
            emit_mlp1(NBLK - 1)
            emit_final(NBLK - 1)

            # ---- transpose p [128, 16] -> [16, 128] and store ----
            ptp = pss.tile([128, 256], dt.float32, space="PSUM", tag="s")
            nc.tensor.transpose(out=ptp[: BPC // 128, :128], in_=p_sb[:],
                                identity=idf[:])
            pout = cpool.tile([BPC // 128, 128], dt.float32)
            nc.vector.tensor_copy(pout[:], ptp[: BPC // 128, :128])
            nc.sync.dma_start(out_d[:].rearrange("(a b) -> a b", b=128),
                              pout[:])

    nc.compile()
    return nc


def _prep_host(inputs):
    """Host-side preprocessing -> per-core input maps."""
    x = np.asarray(inputs["x"]).astype(np.int64)
    tab = np.asarray(inputs["emb_table"], dtype=np.float32)
    k = int(np.asarray(inputs["k"]))

    s_f = (np.asarray(inputs["bn_gamma"], np.float64)
           / np.sqrt(np.asarray(inputs["bn_var"], np.float64) + EPS))
    t_f = np.asarray(inputs["bn_beta"], np.float64) - np.asarray(
        inputs["bn_mean"], np.float64) * s_f
    # table carries only the BN scale; offsets go through T0/Tu/c_ctrl
    tab_raw = tab.astype(np.float64) * np.repeat(s_f, VOCAB)[:, None]
    tab_h = np.zeros((F * VOCAB, 128), bf16)
    tab_h[:, :E] = tab_raw.astype(bf16)

    ctrl_w = np.asarray(inputs["ctrl_w"], np.float64)  # [D, F]
    wc = np.zeros((KT * 128, F), np.float32)
    wc[:D] = ctrl_w.astype(np.float32)
    wc_h = np.ascontiguousarray(
        wc.reshape(KT, 128, F).transpose(1, 0, 2).reshape(128, KT * F)).astype(bf16)
    # ctrl bias + exact offset contribution: c_ctrl[j] = sum_f t_f*sum_e Wc
    c_ctrl = (t_f[:, None] * ctrl_w.reshape(F, E, F).sum(axis=1)).sum(axis=0)
    cb_h = (np.asarray(inputs["ctrl_b"], np.float64) + c_ctrl).astype(
        np.float32).reshape(F, 1)

    # MLP0 with BN scale folded into columns; fp8 DoubleRow plane-major pack
    g0 = (np.asarray(inputs["mlp_g0"], np.float64)
          / np.sqrt(np.asarray(inputs["mlp_v0"], np.float64) + EPS))
    w0g = np.asarray(inputs["mlp_w0"], np.float64) * g0[None, :]  # [D, H0]
    w0p = np.zeros((KT * 128, H0), np.float32)
    w0p[:D] = (w0g * SCALE_W).astype(np.float32)
    w0_8 = w0p.astype(f8)  # [2560, 1024]
    # layout [128, GD*M0*256]: per (G, m): [sub0 plane 128 | sub1 plane 128]
    w0_h = np.zeros((128, GD * M0 * 256), f8)
    for G in range(GD):
        for m in range(M0):
            base = (G * M0 + m) * 256
            for sub in range(2):
                w0_h[:, base + sub * 128 : base + (sub + 1) * 128] = \
                    w0_8[(2 * G + sub) * 128 : (2 * G + sub + 1) * 128,
                         m * 128 : (m + 1) * 128]
    # offset correction: T0[f, j] = t_f * sum_e w0g[f*64+e, j], pre-scaled
    t0_h = ((t_f[:, None] * w0g.reshape(F, E, H0).sum(axis=1))
            * (SCALE_X * SCALE_W)).astype(bf16)
    b0 = ((np.asarray(inputs["mlp_b0"], np.float64)
           - np.asarray(inputs["mlp_m0"], np.float64)) * g0
          + np.asarray(inputs["mlp_be0"], np.float64)).astype(np.float32)
    b0_h = np.ascontiguousarray(b0.reshape(M0, 128).T)

    g1 = (np.asarray(inputs["mlp_g1"], np.float64)
          / np.sqrt(np.asarray(inputs["mlp_v1"], np.float64) + EPS))
    w1 = np.asarray(inputs["mlp_w1"], np.float32) * g1[None, :].astype(np.float32)
    b1 = ((np.asarray(inputs["mlp_b1"], np.float64)
           - np.asarray(inputs["mlp_m1"], np.float64)) * g1
          + np.asarray(inputs["mlp_be1"], np.float64)).astype(np.float32)
    w1_h = np.ascontiguousarray(
        w1.reshape(KT1, 128, M1, 128).transpose(1, 0, 2, 3)
        .reshape(128, KT1 * M1 * 128)).astype(bf16)
    b1_h = np.ascontiguousarray(b1.reshape(M1, 128).T)

    # U weights: 6 cross rows + lin_w[:D]; fp8 DR plane-major [128, GD*14]
    cross_w = np.asarray(inputs["cross_w"], np.float64)
    cross_b = np.asarray(inputs["cross_b"], np.float64)
    lin_w = np.asarray(inputs["lin_w"], np.float64)
    wu = np.zeros((KT * 128, 7), np.float64)
    wu[:D, :L] = cross_w.T
    wu[:D, 6] = lin_w[:D]
    wu_8 = (wu * SCALE_W).astype(np.float32).astype(f8)
    wu_h = np.zeros((128, GD * 32), f8)
    for G in range(GD):
        for sub in range(2):
            wu_h[:, G * 32 + sub * 16 : G * 32 + sub * 16 + 7] = \
                wu_8[(2 * G + sub) * 128 : (2 * G + sub + 1) * 128, :]
    tu_h = ((t_f[:, None] * wu[:D].reshape(F, E, 7).sum(axis=1))
            * (SCALE_X * SCALE_W)).astype(bf16)

    # expand matrix S [F, KT*128] with the fp8 activation scale baked in
    s = np.zeros((F, KT * 128), np.float32)
    feat = np.arange(KT * 128)
    valid = feat < D
    s[feat[valid] // E, feat[valid]] = SCALE_X
    s_h = s.astype(bf16)

    lw2_h = np.ascontiguousarray(
        lin_w[D:].astype(np.float32).reshape(M1, 128).T).astype(bf16)

    # cross-collapse constants: v_l = beta_l . w_l ; c0 = beta_6 . lin_w_a + b
    beta = np.zeros(D, np.float64)
    v = np.zeros(L, np.float64)
    for l in range(L):
        v[l] = beta @ cross_w[l]
        beta = beta + cross_b[l]
    c0 = float(beta @ lin_w[:D]
               + float(np.asarray(inputs["lin_b"]).ravel()[0]))
    v_consts = tuple(float(t) for t in v)

    def wrap16(jj):
        # idx j at [j % 16, j // 16], replicated over 8 partition groups
        w = jj.reshape(-1, 16).T.astype(np.int16)  # [16, n/16]
        return np.tile(w, (8, 1))  # [128, n/16]

    in_maps = []
    for ci in range(NCORES):
        xs = x[ci * BPC : (ci + 1) * BPC]  # [2048, 39]
        idxs = np.zeros((NBLK, 128, IDXW_TOT), np.int16)
        for blk in range(NBLK):
            rows = xs[blk * BLK : (blk + 1) * BLK]  # [512, 39]
            for g in range(KT):  # even fields (direct), slot g
                f = min(2 * g, 38)
                idxs[blk, :, g * IDXW_F : (g + 1) * IDXW_F] = \
                    wrap16(rows[:, f].copy())
            for g in range(19):  # odd fields (scratch), slot 20+g
                idxs[blk, :, (KT + g) * IDXW_F : (KT + g + 1) * IDXW_F] = \
                    wrap16(rows[:, 2 * g + 1].copy())
        in_maps.append({
            "idxs": idxs,
            "tab": tab_h,
            "wc": wc_h,
            "cb": cb_h,
            "w0": w0_h,
            "t0": t0_h,
            "w1": w1_h,
            "wu": wu_h,
            "tu": tu_h,
            "s": s_h,
            "lw2": lw2_h,
            "b0": b0_h,
            "b1": b1_h,
        })
    return in_maps, k, v_consts, c0


def _scheduled_gather_queues(nc):
    """Walk the scheduled program; return [(key, ordinal, queue)] for
    every dma_gather, in scheduled (program) order."""
    out = []
    cnt = 0
    for f in nc.m.functions:
        for bb in f.blocks:
            for inst in bb.instructions:
                if type(inst).__name__ == "InstDMAGatherAnt":
                    key = nc._gather_insts.get(inst.name)
                    out.append((key, cnt, inst.queue_num))
                    cnt += 1
    return out


def _build_aligned(k, v_consts, c0):
    """Build, then verify the SWDGE queue assignment is consistent with
    the scheduler's mod-8 DMASW semaphore rotation (sem lane = scheduled
    ordinal % 8, each lane locked to one queue). If not, rebuild with
    queue = scheduled ordinal % NQ (fixpoint, few iterations)."""
    queue_map = {}
    for attempt in range(4):
        nc = _build(k, v_consts, c0, queue_map)
        sched = _scheduled_gather_queues(nc)
        lane_lock = {}
        ok = True
        for key, ordinal, q in sched:
            lane = ordinal % 8
            if lane_lock.setdefault(lane, q) != q:
                ok = False
        if ok:
            return nc
        new_map = {key: ordinal % NQ for key, ordinal, q in sched
                   if key is not None}
        if new_map == queue_map:
            return nc  # schedule oscillates; give up realigning
        queue_map = new_map
    return nc


def _get_nc(k, v_consts, c0):
    key = (k, v_consts, c0)
    if key not in _CACHE:
        _CACHE[key] = _build_aligned(k, v_consts, c0)
    return _CACHE[key]


def kernel(**inputs) -> np.ndarray:
    in_maps, k, v_consts, c0 = _prep_host(inputs)
    nc = _get_nc(k, v_consts, c0)
    res = run_bass_kernel_spmd(nc, in_maps, core_ids=list(range(NCORES)))
    out = np.concatenate([res.results[i]["out"] for i in range(NCORES)])
    return out.astype(np.float32)


def run_traced(**inputs):
    """Like kernel() but with tracing enabled; returns (out, results)."""
    in_maps, k, v_consts, c0 = _prep_host(inputs)
    nc = _get_nc(k, v_consts, c0)
    res = run_bass_kernel_spmd(nc, in_maps, core_ids=list(range(NCORES)),
                               trace=True)
    out = np.concatenate([res.results[i]["out"] for i in range(NCORES)])
    return out.astype(np.float32), res
